# revision 3
# baseline (speedup 1.0000x reference)
"""YOLO-style detection decode (nms_detection) on 8 trn2 NeuronCores.

Data-parallel over batch (64 -> 8 images/core). Host packs each core's
slice into ONE flat f32 DRAM tensor in CELL-MAJOR layout: for each scale
(52, 26, 13), a [cells_pad, 256] region where each cell row holds the
255 channels (3 anchors x 85) plus one zero pad col; then small consts
(grid offsets, anchors/416). Cell-major removes every PE transpose from
the device: DMA loads are fully contiguous 1KB-per-cell strips and all
compute runs directly on SBUF.

Device pipeline per super-group of S chunks (chunk = 128 cells):
  - one contiguous DMA  -> SBUF [128, S, 256]
  - DVE tensor_reduce (4D view) -> per (cell, anchor) class max (exact)
  - DVE max_index per (chunk, anchor): first index of the max in the
    80-wide class window == exact jnp.argmax (ties -> first)
  - ACT: exp(-conf), +1 (Copy bias), exp(w/h), u32->f32 idx cast
  - DVE: reciprocal (sigmoid finish), cx/cy fused mul-add with host
    grid, w/h mul by anchors/416, conf>0 mask applied multiplicatively
  - one DMA out of [128, S, 18]
"""

import os

import numpy as np

import concourse.bass as bass
import concourse.tile as tile
from concourse import bacc, mybir
from concourse.bass_utils import run_bass_kernel_spmd

N_CORES = 8
B = 64
B_PER = B // N_CORES
CASE = 416.0
SCALES = [("52", 52, 8.0), ("26", 26, 16.0), ("13", 13, 32.0)]
CHUNK = 128
NCH = 256          # padded channels per cell (255 real + 1 zero)
SG = int(os.environ.get("KSG", "16"))   # chunks per super-group

F32 = mybir.dt.float32
U32 = mybir.dt.uint32
AX = mybir.AxisListType
OP = mybir.AluOpType
AF = mybir.ActivationFunctionType


def _cells(h):
    return B_PER * h * h


def _nchunks(h):
    return (_cells(h) + CHUNK - 1) // CHUNK


NCH_TOTAL = sum(_nchunks(h) for _, h, _ in SCALES)   # 223

# packed input layout (f32 elements, per core)
_X_OFF = {}
_off = 0
for _tag, _h, _t in SCALES:
    _X_OFF[_tag] = _off
    _off += _nchunks(_h) * CHUNK * NCH
_GXY_OFF = _off
_off += 128 * 2 * NCH_TOTAL
_ANCH_OFF = _off
_off += 128 * 18
TOTAL_IN = _off

_O_OFF = {}
_off = 0
for _tag, _h, _t in SCALES:
    _O_OFF[_tag] = _off
    _off += _nchunks(_h) * CHUNK
TOTAL_OUT_ROWS = _off    # 28544 (incl. padded tail rows per scale)


def _gxy_section(h, t):
    nch = _nchunks(h)
    n = _cells(h)
    hw = h * h
    cells = np.arange(nch * CHUNK)
    s = cells % hw
    gx = (s % h).astype(np.float64) * t / CASE
    gy = (s // h).astype(np.float64) * t / CASE
    gx[cells >= n] = 0.0
    gy[cells >= n] = 0.0
    out = np.zeros((CHUNK, 2 * nch), np.float32)
    for j in range(nch):
        out[:, 2 * j] = gx[j * CHUNK:(j + 1) * CHUNK]
        out[:, 2 * j + 1] = gy[j * CHUNK:(j + 1) * CHUNK]
    return out


_GXY = np.concatenate([_gxy_section(h, t) for _, h, t in SCALES], axis=1)


def _emit_scale(nc, pools, sb, xin, oX, h, t, tag, chunk_base):
    nch = _nchunks(h)
    k = float(t / CASE)
    p_in, p_out, p_sm = pools
    xoff = _X_OFF[tag]
    ooff = _O_OFF[tag]
    anch_v = sb["anch"]          # [128, 3, 2] view for this scale
    gxy_all = sb["gxy"]          # [128, 2*NCH_TOTAL]

    j0 = 0
    while j0 < nch:
        G = min(SG, nch - j0)
        xt = p_in.tile([128, SG * NCH], F32, tag="x")
        xv = xt[:].rearrange("p (s c) -> p s c", c=NCH)[:, 0:G, :]
        src = xin[xoff + j0 * CHUNK * NCH:
                  xoff + (j0 + G) * CHUNK * NCH] \
            .rearrange("(s p c) -> p s c", p=CHUNK, c=NCH)
        nc.sync.dma_start(xv, src)

        o85 = xv[:, :, 0:255].rearrange("p s (a r) -> p s a r", a=3, r=85)
        cls = o85[:, :, :, 5:85]

        m = p_sm.tile([128, 3 * SG], F32, tag="m")
        mv = m[:].rearrange("p (s a) -> p s a", a=3)[:, 0:G, :]
        nc.vector.tensor_reduce(mv, cls, axis=AX.X, op=OP.max)

        idxu = p_sm.tile([128, 8 * 3 * SG], U32, tag="idxu")
        for s in range(G):
            for a in range(3):
                j = s * 3 + a
                nc.vector.max_index(
                    idxu[:, 8 * j:8 * j + 8],
                    m[:, j:j + 1].broadcast_to([128, 8]),
                    cls[:, s, a, :])

        econf = p_sm.tile([128, 3 * SG], F32, tag="econf")
        ev = econf[:].rearrange("p (s a) -> p s a", a=3)[:, 0:G, :]
        nc.scalar.activation(ev, o85[:, :, :, 0:1].squeeze(3),
                             AF.Exp, scale=-1.0)
        e1 = p_sm.tile([128, 3 * SG], F32, tag="e1")
        e1v = e1[:].rearrange("p (s a) -> p s a", a=3)[:, 0:G, :]
        nc.scalar.activation(e1v, ev, AF.Copy, bias=1.0)

        o4 = p_out.tile([128, 18 * SG], F32, tag="o4")
        ov = o4[:].rearrange("p (s a c) -> p s a c", a=3, c=6)[:, 0:G]
        o18 = o4[:].rearrange("p (s x) -> p s x", x=18)[:, 0:G, :]
        nc.vector.reciprocal(ov[:, :, :, 0:1].squeeze(3), e1v)

        gxy_v = gxy_all[:, 2 * (chunk_base + j0):2 * (chunk_base + j0 + G)] \
            .rearrange("p (s q) -> p s q", q=2)
        twh = p_sm.tile([128, 6 * SG], F32, tag="twh")
        tw6 = twh[:].rearrange("p (s x) -> p s x", x=6)[:, 0:G, :]
        for a in range(3):
            nc.vector.scalar_tensor_tensor(
                o18[:, :, 6 * a + 1:6 * a + 3],
                o85[:, :, a, 1:3], k, gxy_v,
                op0=OP.mult, op1=OP.add)
            nc.scalar.activation(tw6[:, :, 2 * a:2 * a + 2],
                                 o85[:, :, a, 3:5], AF.Exp)
            nc.vector.tensor_tensor(
                o18[:, :, 6 * a + 3:6 * a + 5],
                tw6[:, :, 2 * a:2 * a + 2],
                anch_v[:, a:a + 1, :].broadcast_to([128, G, 2]),
                op=OP.mult)

        iview = idxu[:].rearrange("p (j e) -> p j e", e=8)[:, :, 0:1] \
            .squeeze(2).rearrange("p (s a) -> p s a", a=3)[:, 0:G, :]
        nc.scalar.activation(ov[:, :, :, 5:6].squeeze(3), iview, AF.Copy)

        for a in range(3):
            conf_b = o85[:, :, a, 0:1].broadcast_to([128, G, 6])
            nc.vector.scalar_tensor_tensor(
                o18[:, :, 6 * a:6 * a + 6], conf_b, 0.0,
                o18[:, :, 6 * a:6 * a + 6], op0=OP.is_gt, op1=OP.mult)

        dst = oX[ooff + j0 * CHUNK:ooff + (j0 + G) * CHUNK, :] \
            .rearrange("(s p) c -> p s c", p=CHUNK)
        nc.scalar.dma_start(dst, o18)
        j0 += G


def build():
    nc = bacc.Bacc("TRN2", target_bir_lowering=False, debug=False,
                   num_devices=N_CORES)
    xin = nc.dram_tensor("xin", [TOTAL_IN], F32, kind="ExternalInput").ap()
    oX = nc.dram_tensor("out", [TOTAL_OUT_ROWS, 18], F32,
                        kind="ExternalOutput").ap()

    with tile.TileContext(nc) as tc:
        with tc.tile_pool(name="consts", bufs=1) as p_c, \
                tc.tile_pool(name="inp", bufs=3) as p_in, \
                tc.tile_pool(name="outp", bufs=3) as p_out, \
                tc.tile_pool(name="small", bufs=3) as p_sm:
            gxy_t = p_c.tile([128, 2 * NCH_TOTAL], F32, tag="gxy")
            nc.sync.dma_start(
                gxy_t[:],
                xin[_GXY_OFF:_GXY_OFF + 128 * 2 * NCH_TOTAL]
                .rearrange("(p f) -> p f", p=128))
            anch_t = p_c.tile([128, 18], F32, tag="anch")
            nc.sync.dma_start(
                anch_t[:],
                xin[_ANCH_OFF:_ANCH_OFF + 128 * 18]
                .rearrange("(p f) -> p f", p=128))

            pools = (p_in, p_out, p_sm)
            chunk_base = 0
            anch_off = 0
            for tag, h, t in SCALES:
                sb = {
                    "gxy": gxy_t[:],
                    "anch": anch_t[:, anch_off:anch_off + 6]
                    .rearrange("p (a q) -> p a q", q=2),
                }
                _emit_scale(nc, pools, sb, xin, oX, h, t, tag, chunk_base)
                chunk_base += _nchunks(h)
                anch_off += 6
    nc.compile()
    return nc


_NC = None


def _get_nc():
    global _NC
    if _NC is None:
        _NC = build()
    return _NC


def _make_anch(anchors):
    anch = np.zeros((128, 18), np.float32)
    off = 0
    for tag, h, _ in SCALES:
        a = anchors[tag].astype(np.float64) / CASE
        for aa in range(3):
            for q in range(2):
                anch[:, off + aa * 2 + q] = a[aa, q]
        off += 6
    return anch


def _pack_core(xs, anch):
    buf = np.zeros(TOTAL_IN, np.float32)
    for tag, h, _ in SCALES:
        n = _cells(h)
        region = buf[_X_OFF[tag]:_X_OFF[tag] + _nchunks(h) * CHUNK * NCH] \
            .reshape(-1, NCH)
        x = np.asarray(xs[tag])      # [B_PER, 255, h, w]
        region[:n, 0:255] = x.transpose(0, 2, 3, 1).reshape(n, 255)
    buf[_GXY_OFF:_GXY_OFF + _GXY.size] = _GXY.ravel()
    buf[_ANCH_OFF:_ANCH_OFF + anch.size] = anch.ravel()
    return buf


def kernel(out13, out26, out52, anchors13, anchors26, anchors52):
    nc = _get_nc()
    xs_all = {"13": np.asarray(out13), "26": np.asarray(out26),
              "52": np.asarray(out52)}
    anchors = {"13": np.asarray(anchors13), "26": np.asarray(anchors26),
               "52": np.asarray(anchors52)}
    anch = _make_anch(anchors)

    in_maps = []
    for i in range(N_CORES):
        xs = {tag: xs_all[tag][i * B_PER:(i + 1) * B_PER]
              for tag, _, _ in SCALES}
        in_maps.append({"xin": _pack_core(xs, anch)})

    res = run_bass_kernel_spmd(nc, in_maps, list(range(N_CORES))).results

    parts = []
    for tag, h, _ in SCALES[::-1]:   # output order: 13, 26, 52
        o0 = _O_OFF[tag]
        n = _cells(h)
        for i in range(N_CORES):
            parts.append(res[i]["out"][o0:o0 + n].reshape(-1, 6))
    return np.concatenate(parts, axis=0)


# revision 8
# speedup vs baseline: 175.1004x; 175.1004x over previous
"""YOLO-style detection decode (nms_detection) on 8 trn2 NeuronCores.

Data-parallel over batch (64 -> 8 images/core). Host packs each core's
slice into ONE flat f32 DRAM tensor in CELL-MAJOR layout: for each scale
(52, 26, 13), a [cells_pad, 256] region where each cell row holds the
255 channels (3 anchors x 85) plus one zero pad col; then small consts
(grid offsets, anchors/416). Cell-major removes every PE transpose from
the device: DMA loads are fully contiguous 1KB-per-cell strips and all
compute runs directly on SBUF.

Device pipeline per super-group of S chunks (chunk = 128 cells):
  - one contiguous DMA  -> SBUF [128, S, 256]
  - DVE tensor_reduce (4D view) -> per (cell, anchor) class max (exact)
  - DVE max_index per (chunk, anchor): first index of the max in the
    80-wide class window == exact jnp.argmax (ties -> first)
  - ACT: exp(-conf), +1 (Copy bias), exp(w/h), u32->f32 idx cast
  - DVE: reciprocal (sigmoid finish), cx/cy fused mul-add with host
    grid, w/h mul by anchors/416, conf>0 mask applied multiplicatively
  - one DMA out of [128, S, 18]
"""

import os

import numpy as np

import concourse.bass as bass
import concourse.tile as tile
from concourse import bacc, mybir
from concourse.bass_utils import run_bass_kernel_spmd

N_CORES = 8
B = 64
B_PER = B // N_CORES
CASE = 416.0
SCALES = [("52", 52, 8.0), ("26", 26, 16.0), ("13", 13, 32.0)]
CHUNK = 128
NCH = 256          # padded channels per cell (255 real + 1 zero)
SG = int(os.environ.get("KSG", "16"))   # chunks per super-group

F32 = mybir.dt.float32
U32 = mybir.dt.uint32
AX = mybir.AxisListType
OP = mybir.AluOpType
AF = mybir.ActivationFunctionType


def _cells(h):
    return B_PER * h * h


def _nchunks(h):
    return (_cells(h) + CHUNK - 1) // CHUNK


NCH_TOTAL = sum(_nchunks(h) for _, h, _ in SCALES)   # 223

# packed input layout (f32 elements, per core)
_X_OFF = {}
_off = 0
for _tag, _h, _t in SCALES:
    _X_OFF[_tag] = _off
    _off += _nchunks(_h) * CHUNK * NCH
_GXY_OFF = _off
_off += 128 * 2 * NCH_TOTAL
_ANCH_OFF = _off
_off += 128 * 18
TOTAL_IN = _off

_O_OFF = {}
_off = 0
for _tag, _h, _t in SCALES:
    _O_OFF[_tag] = _off
    _off += _nchunks(_h) * CHUNK
TOTAL_OUT_ROWS = _off    # 28544 (incl. padded tail rows per scale)


def _gxy_section(h, t):
    """Grid offsets packed to match the device cell->partition mapping:
    within a super-group starting at chunk j0 with G chunks, partition p
    holds cells j0*128 + p*G + s for s in 0..G-1 (layout B: consecutive
    cells per partition -> G*1KB contiguous DMA descriptors)."""
    nch = _nchunks(h)
    n = _cells(h)
    hw = h * h
    cells = np.arange(nch * CHUNK)
    s = cells % hw
    gx = (s % h).astype(np.float64) * t / CASE
    gy = (s // h).astype(np.float64) * t / CASE
    gx[cells >= n] = 0.0
    gy[cells >= n] = 0.0
    out = np.zeros((CHUNK, 2 * nch), np.float32)
    j0 = 0
    while j0 < nch:
        G = min(SG, nch - j0)
        for p in range(CHUNK):
            base = j0 * CHUNK + p * G
            out[p, 2 * j0:2 * (j0 + G):2] = gx[base:base + G]
            out[p, 2 * j0 + 1:2 * (j0 + G):2] = gy[base:base + G]
        j0 += G
    return out


_GXY = np.concatenate([_gxy_section(h, t) for _, h, t in SCALES], axis=1)


def _emit_scale(nc, pools, sb, xin, oX, h, t, tag, chunk_base):
    ST = int(os.environ.get("KSTAGE", "9"))
    nch = _nchunks(h)
    k = float(t / CASE)
    p_in, p_out, p_sm = pools
    xoff = _X_OFF[tag]
    ooff = _O_OFF[tag]
    anch_v = sb["anch"]          # [128, 3, 2] view for this scale
    gxy_all = sb["gxy"]          # [128, 2*NCH_TOTAL]

    j0 = 0
    while j0 < nch:
        G = min(SG, nch - j0)
        xt = p_in.tile([128, SG * NCH], F32, tag="x")
        xv = xt[:].rearrange("p (s c) -> p s c", c=NCH)[:, 0:G, :]
        src = xin[xoff + j0 * CHUNK * NCH:
                  xoff + (j0 + G) * CHUNK * NCH] \
            .rearrange("(p s c) -> p s c", p=CHUNK, c=NCH)
        nc.sync.dma_start(xv, src)

        o85 = xv[:, :, 0:255].rearrange("p s (a r) -> p s a r", a=3, r=85)
        cls = o85[:, :, :, 5:85]

        m = p_sm.tile([128, 3 * SG], F32, tag="m")
        mv = m[:].rearrange("p (s a) -> p s a", a=3)[:, 0:G, :]
        if ST >= 2:
            nc.vector.tensor_reduce(mv, cls, axis=AX.X, op=OP.max)
        else:
            nc.vector.memset(m[:], 1.0)

        idxu = p_sm.tile([128, 8 * 3 * SG], U32, tag="idxu")
        if ST < 3:
            nc.vector.memset(idxu[:].bitcast(F32), 0.0)
        for s in range(G if ST >= 3 else 0):
            for a in range(3):
                j = s * 3 + a
                nc.vector.max_index(
                    idxu[:, 8 * j:8 * j + 8],
                    m[:, j:j + 1].broadcast_to([128, 8]),
                    cls[:, s, a, :])

        econf = p_sm.tile([128, 3 * SG], F32, tag="econf")
        ev = econf[:].rearrange("p (s a) -> p s a", a=3)[:, 0:G, :]
        e1 = p_sm.tile([128, 3 * SG], F32, tag="e1")
        e1v = e1[:].rearrange("p (s a) -> p s a", a=3)[:, 0:G, :]
        if ST >= 4:
            nc.scalar.activation(ev, o85[:, :, :, 0:1].squeeze(3),
                                 AF.Exp, scale=-1.0)
            nc.scalar.activation(e1v, ev, AF.Copy, bias=1.0)
        else:
            nc.vector.memset(e1[:], 1.0)

        o4 = p_out.tile([128, 18 * SG], F32, tag="o4")
        ov = o4[:].rearrange("p (s a c) -> p s a c", a=3, c=6)[:, 0:G]
        o18 = o4[:].rearrange("p (s x) -> p s x", x=18)[:, 0:G, :]
        if ST < 7:
            nc.vector.memset(o4[:], 0.0)
        if ST >= 5:
            nc.vector.reciprocal(ov[:, :, :, 0:1].squeeze(3), e1v)

        gxy_v = gxy_all[:, 2 * (chunk_base + j0):2 * (chunk_base + j0 + G)] \
            .rearrange("p (s q) -> p s q", q=2)
        twh = p_sm.tile([128, 6 * SG], F32, tag="twh")
        tw6 = twh[:].rearrange("p (s x) -> p s x", x=6)[:, 0:G, :]
        for a in range(3 if ST >= 5 else 0):
            nc.vector.scalar_tensor_tensor(
                o18[:, :, 6 * a + 1:6 * a + 3],
                o85[:, :, a, 1:3], k, gxy_v,
                op0=OP.mult, op1=OP.add)
            nc.scalar.activation(tw6[:, :, 2 * a:2 * a + 2],
                                 o85[:, :, a, 3:5], AF.Exp)
            nc.vector.tensor_tensor(
                o18[:, :, 6 * a + 3:6 * a + 5],
                tw6[:, :, 2 * a:2 * a + 2],
                anch_v[:, a:a + 1, :].broadcast_to([128, G, 2]),
                op=OP.mult)

        if ST >= 6:
            iview = idxu[:].rearrange("p (j e) -> p j e", e=8)[:, :, 0:1] \
                .squeeze(2).rearrange("p (s a) -> p s a", a=3)[:, 0:G, :]
            nc.scalar.activation(ov[:, :, :, 5:6].squeeze(3), iview, AF.Copy)

        for a in range(3 if ST >= 7 else 0):
            conf_b = o85[:, :, a, 0:1].broadcast_to([128, G, 6])
            nc.vector.scalar_tensor_tensor(
                o18[:, :, 6 * a:6 * a + 6], conf_b, 0.0,
                o18[:, :, 6 * a:6 * a + 6], op0=OP.is_gt, op1=OP.mult)

        dst = oX[ooff + j0 * CHUNK:ooff + (j0 + G) * CHUNK, :] \
            .rearrange("(p s) c -> p s c", p=CHUNK)
        nc.scalar.dma_start(dst, o18)
        j0 += G


def build():
    nc = bacc.Bacc("TRN2", target_bir_lowering=False, debug=False,
                   num_devices=N_CORES)
    xin = nc.dram_tensor("xin", [TOTAL_IN], F32, kind="ExternalInput").ap()
    oX = nc.dram_tensor("out", [TOTAL_OUT_ROWS, 18], F32,
                        kind="ExternalOutput").ap()

    with tile.TileContext(nc) as tc:
        with tc.tile_pool(name="consts", bufs=1) as p_c, \
                tc.tile_pool(name="inp", bufs=3) as p_in, \
                tc.tile_pool(name="outp", bufs=3) as p_out, \
                tc.tile_pool(name="small", bufs=3) as p_sm:
            gxy_t = p_c.tile([128, 2 * NCH_TOTAL], F32, tag="gxy")
            nc.sync.dma_start(
                gxy_t[:],
                xin[_GXY_OFF:_GXY_OFF + 128 * 2 * NCH_TOTAL]
                .rearrange("(p f) -> p f", p=128))
            anch_t = p_c.tile([128, 18], F32, tag="anch")
            nc.sync.dma_start(
                anch_t[:],
                xin[_ANCH_OFF:_ANCH_OFF + 128 * 18]
                .rearrange("(p f) -> p f", p=128))

            pools = (p_in, p_out, p_sm)
            for _rep in range(int(os.environ.get("KREP", "1"))):
                chunk_base = 0
                anch_off = 0
                for tag, h, t in SCALES:
                    sb = {
                        "gxy": gxy_t[:],
                        "anch": anch_t[:, anch_off:anch_off + 6]
                        .rearrange("p (a q) -> p a q", q=2),
                    }
                    _emit_scale(nc, pools, sb, xin, oX, h, t, tag,
                                chunk_base)
                    chunk_base += _nchunks(h)
                    anch_off += 6
    nc.compile()
    return nc


_NC = None


def _get_nc():
    global _NC
    if _NC is None:
        _NC = build()
    return _NC


def _make_anch(anchors):
    anch = np.zeros((128, 18), np.float32)
    off = 0
    for tag, h, _ in SCALES:
        a = anchors[tag].astype(np.float64) / CASE
        for aa in range(3):
            for q in range(2):
                anch[:, off + aa * 2 + q] = a[aa, q]
        off += 6
    return anch


def _pack_core(xs, anch):
    buf = np.zeros(TOTAL_IN, np.float32)
    for tag, h, _ in SCALES:
        n = _cells(h)
        region = buf[_X_OFF[tag]:_X_OFF[tag] + _nchunks(h) * CHUNK * NCH] \
            .reshape(-1, NCH)
        x = np.asarray(xs[tag])      # [B_PER, 255, h, w]
        region[:n, 0:255] = x.transpose(0, 2, 3, 1).reshape(n, 255)
    buf[_GXY_OFF:_GXY_OFF + _GXY.size] = _GXY.ravel()
    buf[_ANCH_OFF:_ANCH_OFF + anch.size] = anch.ravel()
    return buf


def kernel(out13, out26, out52, anchors13, anchors26, anchors52):
    nc = _get_nc()
    xs_all = {"13": np.asarray(out13), "26": np.asarray(out26),
              "52": np.asarray(out52)}
    anchors = {"13": np.asarray(anchors13), "26": np.asarray(anchors26),
               "52": np.asarray(anchors52)}
    anch = _make_anch(anchors)

    in_maps = []
    for i in range(N_CORES):
        xs = {tag: xs_all[tag][i * B_PER:(i + 1) * B_PER]
              for tag, _, _ in SCALES}
        in_maps.append({"xin": _pack_core(xs, anch)})

    res = run_bass_kernel_spmd(nc, in_maps, list(range(N_CORES))).results

    parts = []
    for tag, h, _ in SCALES[::-1]:   # output order: 13, 26, 52
        o0 = _O_OFF[tag]
        n = _cells(h)
        for i in range(N_CORES):
            parts.append(res[i]["out"][o0:o0 + n].reshape(-1, 6))
    return np.concatenate(parts, axis=0)
